# revision 18
# baseline (speedup 1.0000x reference)
"""DeStationaryAttention Trainium2 kernel.

Full inputs in, full output out. Sharding: B*N = 64 attention heads are
split across 8 NeuronCores, 8 heads each: core c handles batch b = c//2,
nodes n0 = (c%2)*8 .. n0+8. Inputs are pre-sliced on the host so each
core receives contiguous [T=1024, H=8, D=128] tensors.

Per-head math (T=1024, D=128):
    Qc = Q - mean_T(Q)
    tau = 2*sigmoid(mean_T(std)*w + b)          (scalar per head)
    S[t,s] = Qc[t]·Kc[s] / sqrt(D)
    out = softmax(tau*S) @ V
K-centering is dropped: softmax_s(Qc·(K-muK)) == softmax_s(Qc·K) because
the Qc[t]·muK term is constant along s. Exponent args are bounded (|.| ≲ 10)
so no max-subtraction is needed in fp32.

Device layout per head:
    qT,kT  = [D=128 part, T free] via PE transposes (is_transpose mode)
    S^T    = kT_slice.T @ qcT  (fp32r matmuls, N=512 -> full PE rate)
    E^T    = exp(tau_scale * S^T) on ScalarE (PSUM -> SBUF)
    O^T   += V_nat_slice.T @ E^T  (fp32r, accumulated in PSUM)
    rowsum = per-t-tile mini-matmuls over Esum = sum_i E^T_i
    out    = PE-transpose(O^T) * (1/rowsum)  -> natural [T,D] -> HBM
"""

import os
import sys
from contextlib import ExitStack

for _p in ("/root/.axon_site/_ro/trn_rl_repo", "/opt/trn_rl_repo"):
    if os.path.isdir(_p) and _p not in sys.path:
        sys.path.append(_p)

import numpy as np

import concourse.bass as bass
import concourse.mybir as mybir
import concourse.tile as tile
from concourse import bacc
from concourse.bass_utils import run_bass_kernel_spmd
from concourse.masks import make_identity

B, T, N, D = 4, 1024, 16, 128
H = 8           # heads per core
NCORES = 8
TT = T // 128   # 128-row tiles along T
F32 = mybir.dt.float32
F32R = mybir.dt.float32r
SCALE2 = 2.0 * D ** (-0.5)   # folded 2*sigmoid(...) * D^-0.5 broadcast constant


def _r(ap):
    return ap.bitcast(F32R)


def _emit(tc):
    nc = tc.nc
    q_d = nc.dram_tensor("Q", [T, H, D], F32, kind="ExternalInput").ap()
    k_d = nc.dram_tensor("K", [T, H, D], F32, kind="ExternalInput").ap()
    v_d = nc.dram_tensor("V", [T, H, D], F32, kind="ExternalInput").ap()
    std_d = nc.dram_tensor("S", [T, H], F32, kind="ExternalInput").ap()
    tw_d = nc.dram_tensor("TW", [1, 1], F32, kind="ExternalInput").ap()
    tb_d = nc.dram_tensor("TB", [1, 1], F32, kind="ExternalInput").ap()
    o_d = nc.dram_tensor("O", [T, H, D], F32, kind="ExternalOutput").ap()

    Exp = mybir.ActivationFunctionType.Exp
    X = mybir.AxisListType.X

    ctx = ExitStack()
    const = ctx.enter_context(tc.tile_pool(name="const", bufs=1))
    nat = ctx.enter_context(tc.tile_pool(name="nat", bufs=2))
    big = ctx.enter_context(tc.tile_pool(name="big", bufs=2))
    etp = ctx.enter_context(tc.tile_pool(name="etp", bufs=4))
    esp = ctx.enter_context(tc.tile_pool(name="esp", bufs=2))
    otsp = ctx.enter_context(tc.tile_pool(name="otsp", bufs=2))
    onatp = ctx.enter_context(tc.tile_pool(name="onatp", bufs=2))
    smallp = ctx.enter_context(tc.tile_pool(name="smallp", bufs=3))
    ps_st = ctx.enter_context(tc.tile_pool(name="ps_st", bufs=2, space="PSUM"))
    ps_ot = ctx.enter_context(tc.tile_pool(name="ps_ot", bufs=1, space="PSUM"))
    ps_sm = ctx.enter_context(tc.tile_pool(name="ps_sm", bufs=2, space="PSUM"))

    # constants
    ident = const.tile([128, 128], F32)
    make_identity(nc, ident)
    ones128 = const.tile([128, 1], F32)
    nc.vector.memset(ones128, 1.0)
    inv_t = const.tile([128, 1], F32)
    nc.vector.memset(inv_t, 1.0 / T)
    bc2 = const.tile([1, 128], F32)
    nc.vector.memset(bc2, SCALE2)

    std_sb = const.tile([128, T * H // 128], F32)   # [128, 64] contiguous
    nc.sync.dma_start(out=std_sb, in_=std_d.rearrange("(p j) h -> p (j h)", p=128))
    tw_sb = const.tile([1, 1], F32)
    nc.sync.dma_start(out=tw_sb, in_=tw_d)
    tb_sb = const.tile([1, 1], F32)
    nc.sync.dma_start(out=tb_sb, in_=tb_d)
    negw = const.tile([1, 1], F32)
    nc.vector.tensor_scalar_mul(negw, tw_sb, -1.0)
    negb = const.tile([1, 1], F32)
    nc.vector.tensor_scalar_mul(negb, tb_sb, -1.0)

    std3 = std_sb.rearrange("p (j h) -> p j h", h=H)

    for h in range(H):
        # ---- loads (natural [t_mod, tt, d] tiling) ----
        q_nat = nat.tile([128, TT, 128], F32, tag="q_nat")
        nc.sync.dma_start(out=q_nat, in_=q_d[:, h, :].rearrange("(tt p) d -> p tt d", p=128))
        k_nat = nat.tile([128, TT, 128], F32, tag="k_nat")
        nc.sync.dma_start(out=k_nat, in_=k_d[:, h, :].rearrange("(tt p) d -> p tt d", p=128))
        v_nat = nat.tile([128, TT, 128], F32R, tag="v_nat")
        nc.sync.dma_start(out=v_nat, in_=_r(v_d[:, h, :].rearrange("(tt p) d -> p tt d", p=128)))

        # ---- tau_scale = 2/sqrt(D) * sigmoid(mean(std)*w + b), as [128,1] ----
        part = smallp.tile([128, 1], F32, tag="part")
        nc.vector.reduce_sum(out=part, in_=std3[:, :, h], axis=X)
        mean_ps = ps_sm.tile([1, 1], F32, tag="ps_sm")
        nc.tensor.matmul(mean_ps, lhsT=inv_t, rhs=part, start=True, stop=True)
        ez = smallp.tile([1, 1], F32, tag="ez")
        nc.scalar.activation(ez, mean_ps, Exp, bias=negb[:], scale=negw[:])
        den = smallp.tile([1, 1], F32, tag="den")
        nc.vector.tensor_scalar_add(den, ez, 1.0)
        sig = smallp.tile([1, 1], F32, tag="sig")
        nc.vector.reciprocal(sig, den)
        tau_ps = ps_sm.tile([128, 1], F32, tag="ps_sm")
        nc.tensor.matmul(tau_ps, lhsT=bc2, rhs=sig, start=True, stop=True)
        tau_sc = smallp.tile([128, 1], F32, tag="tau_sc")
        nc.vector.tensor_copy(tau_sc, tau_ps)

        # ---- transposes: qT,kT = [d, t] (kT rounded to f32r on evacuation) ----
        qT = big.tile([128, T], F32, tag="qT")
        kT = big.tile([128, T], F32R, tag="kT")
        for dst, src in ((qT, q_nat), (kT, k_nat)):
            for a in range(TT // 4):
                pack = ps_sm.tile([128, 512], F32, tag="ps_sm")
                for j in range(4):
                    nc.tensor.transpose(pack[:, j * 128:(j + 1) * 128], src[:, a * 4 + j, :], ident)
                nc.vector.tensor_copy(dst[:, a * 512:(a + 1) * 512], pack)

        # ---- center Q: qcT = qT - mean_t(qT), rounded to f32r ----
        mu = smallp.tile([128, 1], F32, tag="mu")
        nc.vector.reduce_sum(out=mu, in_=qT, axis=X)
        nmu = smallp.tile([128, 1], F32, tag="nmu")
        nc.vector.tensor_scalar_mul(nmu, mu, -1.0 / T)
        qcT = big.tile([128, T], F32R, tag="qcT")
        nc.vector.tensor_scalar_add(qcT, qT, nmu[:])

        # ---- main s-tile loop ----
        ot_ps = ps_ot.tile([128, T], F32, tag="ps_ot")
        esum = esp.tile([128, T], F32, tag="esum")
        esum2 = esp.tile([128, T], F32, tag="esum2")
        for i in range(TT):
            st_ps = ps_st.tile([128, T], F32, tag="ps_st")
            klhs = kT[:, i * 128:(i + 1) * 128]
            nc.tensor.matmul(st_ps[:, 0:512], lhsT=klhs, rhs=qcT[:, 0:512], start=True, stop=True)
            nc.tensor.matmul(st_ps[:, 512:1024], lhsT=klhs, rhs=qcT[:, 512:1024], start=True, stop=True)
            et = etp.tile([128, T], F32R, tag="et")
            nc.scalar.activation(et, st_ps, Exp, bias=0.0, scale=tau_sc[:])
            vlhs = v_nat[:, i, :]
            nc.tensor.matmul(ot_ps[:, 0:512], lhsT=vlhs, rhs=et[:, 0:512], start=(i == 0), stop=(i == TT - 1))
            nc.tensor.matmul(ot_ps[:, 512:1024], lhsT=vlhs, rhs=et[:, 512:1024], start=(i == 0), stop=(i == TT - 1))
            # esum accumulation: two independent partial chains so the DVE and
            # GpSimd halves don't serialize on one accumulator.
            if i == 1:
                nc.vector.tensor_add(esum, prev_et.bitcast(F32), et.bitcast(F32))
            elif i in (2, 3):
                nc.vector.tensor_add(esum, esum, et.bitcast(F32))
            elif i == 5:
                nc.gpsimd.tensor_add(esum2, prev_et.bitcast(F32), et.bitcast(F32))
            elif i in (6, 7):
                nc.gpsimd.tensor_add(esum2, esum2, et.bitcast(F32))
            prev_et = et

        # ---- rowsums (transposed orientation) + reciprocal ----
        nc.vector.tensor_add(esum, esum, esum2)
        rs_ps = ps_sm.tile([128, TT], F32, tag="ps_sm")
        for tt in range(TT):
            nc.tensor.matmul(rs_ps[:, tt:tt + 1], lhsT=esum[:, tt * 128:(tt + 1) * 128],
                             rhs=ones128, start=True, stop=True)
        recipT = smallp.tile([128, TT], F32, tag="recipT")
        nc.vector.reciprocal(recipT, rs_ps)

        # ---- evacuate O^T, transpose back, normalize ----
        ots = otsp.tile([128, T], F32, tag="ots")
        nc.vector.tensor_copy(ots, ot_ps)
        o_nat = onatp.tile([128, TT, 128], F32, tag="o_nat")
        for a in range(TT // 4):
            fpack = ps_sm.tile([128, 512], F32, tag="ps_sm")
            for j in range(4):
                tt = a * 4 + j
                nc.tensor.transpose(fpack[:, j * 128:(j + 1) * 128], ots[:, tt * 128:(tt + 1) * 128], ident)
            for j in range(4):
                tt = a * 4 + j
                nc.vector.tensor_scalar_mul(o_nat[:, tt, :], fpack[:, j * 128:(j + 1) * 128],
                                            recipT[:, tt:tt + 1])
        nc.sync.dma_start(out=o_d[:, h, :].rearrange("(tt p) d -> p tt d", p=128), in_=o_nat)

    ctx.close()


_BUILT = None


def _build():
    global _BUILT
    if _BUILT is None:
        nc = bacc.Bacc("TRN2", target_bir_lowering=False, debug=False, num_devices=None)
        with tile.TileContext(nc) as tc:
            _emit(tc)
        nc.compile()
        _BUILT = nc
    return _BUILT


def _in_maps(Q, K, V, std, tau_w, tau_b):
    tw = np.asarray(tau_w, np.float32).reshape(1, 1)
    tb = np.asarray(tau_b, np.float32).reshape(1, 1)
    maps = []
    for c in range(NCORES):
        b, n0 = c // 2, (c % 2) * H
        maps.append({
            "Q": np.ascontiguousarray(Q[b, :, n0:n0 + H, :], np.float32),
            "K": np.ascontiguousarray(K[b, :, n0:n0 + H, :], np.float32),
            "V": np.ascontiguousarray(V[b, :, n0:n0 + H, :], np.float32),
            "S": np.ascontiguousarray(std[b, :, n0:n0 + H, 0], np.float32),
            "TW": tw,
            "TB": tb,
        })
    return maps


def _gather(results):
    out = np.empty((B, T, N, D), np.float32)
    for c in range(NCORES):
        b, n0 = c // 2, (c % 2) * H
        out[b, :, n0:n0 + H, :] = results[c]["O"]
    return out


def run(Q, K, V, std, tau_w, tau_b, **spmd_kwargs):
    nc = _build()
    res = run_bass_kernel_spmd(nc, _in_maps(Q, K, V, std, tau_w, tau_b),
                               core_ids=list(range(NCORES)), **spmd_kwargs)
    return _gather(res.results), res


def kernel(Q, K, V, std, tau_w, tau_b):
    out, _ = run(Q, K, V, std, tau_w, tau_b)
    return out


# revision 21
# speedup vs baseline: 1.1034x; 1.1034x over previous
"""DeStationaryAttention Trainium2 kernel.

Full inputs in, full output out. Sharding: B*N = 64 attention heads are
split across 8 NeuronCores, 8 heads each: core c handles batch b = c//2,
nodes n0 = (c%2)*8 .. n0+8. Inputs are pre-sliced on the host so each
core receives contiguous [T=1024, H=8, D=128] tensors.

Per-head math (T=1024, D=128):
    Qc = Q - mean_T(Q)
    tau = 2*sigmoid(mean_T(std)*w + b)          (scalar per head)
    S[t,s] = Qc[t]·Kc[s] / sqrt(D)
    out = softmax(tau*S) @ V
K-centering is dropped: softmax_s(Qc·(K-muK)) == softmax_s(Qc·K) because
the Qc[t]·muK term is constant along s. Exponent args are bounded (|.| ≲ 10)
so no max-subtraction is needed in fp32.

Device layout per head:
    qT,kT  = [D=128 part, T free] via PE transposes (is_transpose mode)
    S^T    = kT_slice.T @ qcT  (fp32r matmuls, N=512 -> full PE rate)
    E^T    = exp(tau_scale * S^T) on ScalarE (PSUM -> SBUF)
    O^T   += V_nat_slice.T @ E^T  (fp32r, accumulated in PSUM)
    rowsum = per-t-tile mini-matmuls over Esum = sum_i E^T_i
    out    = PE-transpose(O^T) * (1/rowsum)  -> natural [T,D] -> HBM
"""

import os
import sys
from contextlib import ExitStack

for _p in ("/root/.axon_site/_ro/trn_rl_repo", "/opt/trn_rl_repo"):
    if os.path.isdir(_p) and _p not in sys.path:
        sys.path.append(_p)

import numpy as np

import concourse.bass as bass
import concourse.mybir as mybir
import concourse.tile as tile
from concourse import bacc
from concourse.bass_utils import run_bass_kernel_spmd
from concourse.masks import make_identity

B, T, N, D = 4, 1024, 16, 128
H = 8           # heads per core
NCORES = 8
TT = T // 128   # 128-row tiles along T
F32 = mybir.dt.float32
F32R = mybir.dt.float32r
SCALE2 = 2.0 * D ** (-0.5)   # folded 2*sigmoid(...) * D^-0.5 broadcast constant


def _r(ap):
    return ap.bitcast(F32R)


def _emit(tc):
    nc = tc.nc
    q_d = nc.dram_tensor("Q", [T, H, D], F32, kind="ExternalInput").ap()
    k_d = nc.dram_tensor("K", [T, H, D], F32, kind="ExternalInput").ap()
    v_d = nc.dram_tensor("V", [T, H, D], F32, kind="ExternalInput").ap()
    std_d = nc.dram_tensor("S", [T, H], F32, kind="ExternalInput").ap()
    tw_d = nc.dram_tensor("TW", [1, 1], F32, kind="ExternalInput").ap()
    tb_d = nc.dram_tensor("TB", [1, 1], F32, kind="ExternalInput").ap()
    o_d = nc.dram_tensor("O", [T, H, D], F32, kind="ExternalOutput").ap()

    Exp = mybir.ActivationFunctionType.Exp
    X = mybir.AxisListType.X

    ctx = ExitStack()
    const = ctx.enter_context(tc.tile_pool(name="const", bufs=1))
    nat = ctx.enter_context(tc.tile_pool(name="nat", bufs=2))
    big = ctx.enter_context(tc.tile_pool(name="big", bufs=2))
    etp = ctx.enter_context(tc.tile_pool(name="etp", bufs=4))
    esp = ctx.enter_context(tc.tile_pool(name="esp", bufs=2))
    otsp = ctx.enter_context(tc.tile_pool(name="otsp", bufs=2))
    onatp = ctx.enter_context(tc.tile_pool(name="onatp", bufs=2))
    smallp = ctx.enter_context(tc.tile_pool(name="smallp", bufs=3))
    ps_st = ctx.enter_context(tc.tile_pool(name="ps_st", bufs=2, space="PSUM"))
    ps_ot = ctx.enter_context(tc.tile_pool(name="ps_ot", bufs=1, space="PSUM"))
    ps_sm = ctx.enter_context(tc.tile_pool(name="ps_sm", bufs=2, space="PSUM"))

    # constants
    ident = const.tile([128, 128], F32)
    make_identity(nc, ident)
    ones128 = const.tile([128, 1], F32)
    nc.vector.memset(ones128, 1.0)
    inv_t = const.tile([128, 1], F32)
    nc.vector.memset(inv_t, 1.0 / T)
    bc2 = const.tile([1, 128], F32)
    nc.vector.memset(bc2, SCALE2)

    std_sb = const.tile([128, T * H // 128], F32)   # [128, 64] contiguous
    nc.sync.dma_start(out=std_sb, in_=std_d.rearrange("(p j) h -> p (j h)", p=128))
    tw_sb = const.tile([1, 1], F32)
    nc.sync.dma_start(out=tw_sb, in_=tw_d)
    tb_sb = const.tile([1, 1], F32)
    nc.sync.dma_start(out=tb_sb, in_=tb_d)
    negw = const.tile([1, 1], F32)
    nc.vector.tensor_scalar_mul(negw, tw_sb, -1.0)
    negb = const.tile([1, 1], F32)
    nc.vector.tensor_scalar_mul(negb, tb_sb, -1.0)

    std3 = std_sb.rearrange("p (j h) -> p j h", h=H)

    for h in range(H):
        # ---- loads (natural [t_mod, tt, d] tiling) ----
        q_nat = nat.tile([128, TT, 128], F32, tag="q_nat")
        nc.sync.dma_start(out=q_nat, in_=q_d[:, h, :].rearrange("(tt p) d -> p tt d", p=128))
        k_nat = nat.tile([128, TT, 128], F32, tag="k_nat")
        nc.sync.dma_start(out=k_nat, in_=k_d[:, h, :].rearrange("(tt p) d -> p tt d", p=128))
        v_nat = nat.tile([128, TT, 128], F32R, tag="v_nat")
        nc.sync.dma_start(out=v_nat, in_=_r(v_d[:, h, :].rearrange("(tt p) d -> p tt d", p=128)))

        # ---- tau_scale = 2/sqrt(D) * sigmoid(mean(std)*w + b), as [128,1] ----
        part = smallp.tile([128, 1], F32, tag="part")
        nc.vector.reduce_sum(out=part, in_=std3[:, :, h], axis=X)
        mean_ps = ps_sm.tile([1, 1], F32, tag="ps_sm")
        nc.tensor.matmul(mean_ps, lhsT=inv_t, rhs=part, start=True, stop=True)
        ez = smallp.tile([1, 1], F32, tag="ez")
        nc.scalar.activation(ez, mean_ps, Exp, bias=negb[:], scale=negw[:])
        den = smallp.tile([1, 1], F32, tag="den")
        nc.vector.tensor_scalar_add(den, ez, 1.0)
        sig = smallp.tile([1, 1], F32, tag="sig")
        nc.vector.reciprocal(sig, den)
        tau_ps = ps_sm.tile([128, 1], F32, tag="ps_sm")
        nc.tensor.matmul(tau_ps, lhsT=bc2, rhs=sig, start=True, stop=True)
        tau_sc = smallp.tile([128, 1], F32, tag="tau_sc")
        nc.vector.tensor_copy(tau_sc, tau_ps)

        # ---- transposes: qT,kT = [d, t] (kT rounded to f32r on evacuation) ----
        qT = big.tile([128, T], F32, tag="qT")
        kT = big.tile([128, T], F32R, tag="kT")
        for dst, src in ((qT, q_nat), (kT, k_nat)):
            for a in range(TT // 4):
                pack = ps_sm.tile([128, 512], F32, tag="ps_sm")
                for j in range(4):
                    nc.tensor.transpose(pack[:, j * 128:(j + 1) * 128], src[:, a * 4 + j, :], ident)
                nc.vector.tensor_copy(dst[:, a * 512:(a + 1) * 512], pack)

        # ---- center Q: qcT = qT - mean_t(qT), rounded to f32r ----
        mu = smallp.tile([128, 1], F32, tag="mu")
        nc.vector.reduce_sum(out=mu, in_=qT, axis=X)
        nmu = smallp.tile([128, 1], F32, tag="nmu")
        nc.vector.tensor_scalar_mul(nmu, mu, -1.0 / T)
        qcT = big.tile([128, T], F32R, tag="qcT")
        nc.vector.tensor_scalar_add(qcT, qT, nmu[:])

        # ---- main s-tile loop ----
        ot_ps = ps_ot.tile([128, T], F32, tag="ps_ot")
        esum = esp.tile([128, T], F32, tag="esum")
        for i in range(TT):
            st_ps = ps_st.tile([128, T], F32, tag="ps_st")
            klhs = kT[:, i * 128:(i + 1) * 128]
            nc.tensor.matmul(st_ps[:, 0:512], lhsT=klhs, rhs=qcT[:, 0:512], start=True, stop=True)
            nc.tensor.matmul(st_ps[:, 512:1024], lhsT=klhs, rhs=qcT[:, 512:1024], start=True, stop=True)
            et = etp.tile([128, T], F32R, tag="et")
            nc.scalar.activation(et, st_ps, Exp, bias=0.0, scale=tau_sc[:])
            vlhs = v_nat[:, i, :]
            nc.tensor.matmul(ot_ps[:, 0:512], lhsT=vlhs, rhs=et[:, 0:512], start=(i == 0), stop=(i == TT - 1))
            nc.tensor.matmul(ot_ps[:, 512:1024], lhsT=vlhs, rhs=et[:, 512:1024], start=(i == 0), stop=(i == TT - 1))
            # esum accumulation: two independent partial chains so the DVE and
            # GpSimd halves don't serialize on one accumulator.
            if i == 1:
                nc.vector.tensor_add(esum, prev_et.bitcast(F32), et.bitcast(F32))
            elif i > 1:
                nc.vector.tensor_add(esum, esum, et.bitcast(F32))
            prev_et = et

        # ---- rowsums (transposed orientation) + reciprocal ----
        rs_ps = ps_sm.tile([128, TT], F32, tag="ps_sm")
        for tt in range(TT):
            nc.tensor.matmul(rs_ps[:, tt:tt + 1], lhsT=esum[:, tt * 128:(tt + 1) * 128],
                             rhs=ones128, start=True, stop=True)
        recipT = smallp.tile([128, TT], F32, tag="recipT")
        nc.vector.reciprocal(recipT, rs_ps)

        # ---- evacuate O^T, transpose back, normalize ----
        ots = otsp.tile([128, T], F32, tag="ots")
        nc.vector.tensor_copy(ots, ot_ps)
        o_nat = onatp.tile([128, TT, 128], F32, tag="o_nat")
        for a in range(TT // 4):
            fpack = ps_sm.tile([128, 512], F32, tag="ps_sm")
            for j in range(4):
                tt = a * 4 + j
                nc.tensor.transpose(fpack[:, j * 128:(j + 1) * 128], ots[:, tt * 128:(tt + 1) * 128], ident)
            for j in range(4):
                tt = a * 4 + j
                nc.vector.tensor_scalar_mul(o_nat[:, tt, :], fpack[:, j * 128:(j + 1) * 128],
                                            recipT[:, tt:tt + 1])
        nc.sync.dma_start(out=o_d[:, h, :].rearrange("(tt p) d -> p tt d", p=128), in_=o_nat)

    ctx.close()


_BUILT = None


def _build():
    global _BUILT
    if _BUILT is None:
        nc = bacc.Bacc("TRN2", target_bir_lowering=False, debug=False, num_devices=None)
        with tile.TileContext(nc) as tc:
            _emit(tc)
        nc.compile()
        _BUILT = nc
    return _BUILT


def _in_maps(Q, K, V, std, tau_w, tau_b):
    tw = np.asarray(tau_w, np.float32).reshape(1, 1)
    tb = np.asarray(tau_b, np.float32).reshape(1, 1)
    maps = []
    for c in range(NCORES):
        b, n0 = c // 2, (c % 2) * H
        maps.append({
            "Q": np.ascontiguousarray(Q[b, :, n0:n0 + H, :], np.float32),
            "K": np.ascontiguousarray(K[b, :, n0:n0 + H, :], np.float32),
            "V": np.ascontiguousarray(V[b, :, n0:n0 + H, :], np.float32),
            "S": np.ascontiguousarray(std[b, :, n0:n0 + H, 0], np.float32),
            "TW": tw,
            "TB": tb,
        })
    return maps


def _gather(results):
    out = np.empty((B, T, N, D), np.float32)
    for c in range(NCORES):
        b, n0 = c // 2, (c % 2) * H
        out[b, :, n0:n0 + H, :] = results[c]["O"]
    return out


def run(Q, K, V, std, tau_w, tau_b, **spmd_kwargs):
    nc = _build()
    res = run_bass_kernel_spmd(nc, _in_maps(Q, K, V, std, tau_w, tau_b),
                               core_ids=list(range(NCORES)), **spmd_kwargs)
    return _gather(res.results), res


def kernel(Q, K, V, std, tau_w, tau_b):
    out, _ = run(Q, K, V, std, tau_w, tau_b)
    return out


# revision 22
# speedup vs baseline: 1.2965x; 1.1750x over previous
"""DeStationaryAttention Trainium2 kernel.

Full inputs in, full output out. Sharding: B*N = 64 attention heads are
split across 8 NeuronCores, 8 heads each: core c handles batch b = c//2,
nodes n0 = (c%2)*8 .. n0+8. Inputs are pre-sliced on the host so each
core receives contiguous [T=1024, H=8, D=128] tensors.

Per-head math (T=1024, D=128):
    Qc = Q - mean_T(Q)
    tau = 2*sigmoid(mean_T(std)*w + b)          (scalar per head)
    S[t,s] = Qc[t]·Kc[s] / sqrt(D)
    out = softmax(tau*S) @ V
K-centering is dropped: softmax_s(Qc·(K-muK)) == softmax_s(Qc·K) because
the Qc[t]·muK term is constant along s. Exponent args are bounded (|.| ≲ 10)
so no max-subtraction is needed in fp32.

Device layout per head:
    qT,kT  = [D=128 part, T free] via PE transposes (is_transpose mode)
    S^T    = kT_slice.T @ qcT  (fp32r matmuls, N=512 -> full PE rate)
    E^T    = exp(tau_scale * S^T) on ScalarE (PSUM -> SBUF)
    O^T   += V_nat_slice.T @ E^T  (fp32r, accumulated in PSUM)
    rowsum = per-t-tile mini-matmuls over Esum = sum_i E^T_i
    out    = PE-transpose(O^T) * (1/rowsum)  -> natural [T,D] -> HBM
"""

import os
import sys
from contextlib import ExitStack

for _p in ("/root/.axon_site/_ro/trn_rl_repo", "/opt/trn_rl_repo"):
    if os.path.isdir(_p) and _p not in sys.path:
        sys.path.append(_p)

import numpy as np

import concourse.bass as bass
import concourse.mybir as mybir
import concourse.tile as tile
from concourse import bacc
from concourse.bass_utils import run_bass_kernel_spmd
from concourse.masks import make_identity

B, T, N, D = 4, 1024, 16, 128
H = 8           # heads per core
NCORES = 8
TT = T // 128   # 128-row tiles along T
F32 = mybir.dt.float32
F32R = mybir.dt.float32r
SCALE2 = 2.0 * D ** (-0.5)   # folded 2*sigmoid(...) * D^-0.5 broadcast constant


def _r(ap):
    return ap.bitcast(F32R)


def _emit(tc):
    nc = tc.nc
    q_d = nc.dram_tensor("Q", [T, H, D], F32, kind="ExternalInput").ap()
    k_d = nc.dram_tensor("K", [T, H, D], F32, kind="ExternalInput").ap()
    v_d = nc.dram_tensor("V", [T, H, D], F32, kind="ExternalInput").ap()
    std_d = nc.dram_tensor("S", [T, H], F32, kind="ExternalInput").ap()
    tw_d = nc.dram_tensor("TW", [1, 1], F32, kind="ExternalInput").ap()
    tb_d = nc.dram_tensor("TB", [1, 1], F32, kind="ExternalInput").ap()
    o_d = nc.dram_tensor("O", [T, H, D], F32, kind="ExternalOutput").ap()

    Exp = mybir.ActivationFunctionType.Exp
    X = mybir.AxisListType.X

    ctx = ExitStack()
    const = ctx.enter_context(tc.tile_pool(name="const", bufs=1))
    nat = ctx.enter_context(tc.tile_pool(name="nat", bufs=2))
    big = ctx.enter_context(tc.tile_pool(name="big", bufs=2))
    etp = ctx.enter_context(tc.tile_pool(name="etp", bufs=4))
    esp = ctx.enter_context(tc.tile_pool(name="esp", bufs=2))
    otsp = ctx.enter_context(tc.tile_pool(name="otsp", bufs=2))
    onatp = ctx.enter_context(tc.tile_pool(name="onatp", bufs=2))
    smallp = ctx.enter_context(tc.tile_pool(name="smallp", bufs=3))
    ps_st = ctx.enter_context(tc.tile_pool(name="ps_st", bufs=2, space="PSUM"))
    ps_ot = ctx.enter_context(tc.tile_pool(name="ps_ot", bufs=1, space="PSUM"))
    ps_sm = ctx.enter_context(tc.tile_pool(name="ps_sm", bufs=2, space="PSUM"))

    # constants
    ident = const.tile([128, 128], F32)
    make_identity(nc, ident)
    ones128 = const.tile([128, 1], F32)
    nc.vector.memset(ones128, 1.0)
    inv_t = const.tile([128, 1], F32)
    nc.vector.memset(inv_t, 1.0 / T)
    bc2 = const.tile([1, 128], F32)
    nc.vector.memset(bc2, SCALE2)

    std_sb = const.tile([128, T * H // 128], F32)   # [128, 64] contiguous
    nc.sync.dma_start(out=std_sb, in_=std_d.rearrange("(p j) h -> p (j h)", p=128))
    tw_sb = const.tile([1, 1], F32)
    nc.sync.dma_start(out=tw_sb, in_=tw_d)
    tb_sb = const.tile([1, 1], F32)
    nc.sync.dma_start(out=tb_sb, in_=tb_d)
    negw = const.tile([1, 1], F32)
    nc.vector.tensor_scalar_mul(negw, tw_sb, -1.0)
    negb = const.tile([1, 1], F32)
    nc.vector.tensor_scalar_mul(negb, tb_sb, -1.0)

    std3 = std_sb.rearrange("p (j h) -> p j h", h=H)
    Ident = mybir.ActivationFunctionType.Identity

    # ---- prologue: tau_scale[h] = 2/sqrt(D) * sigmoid(mean(std_h)*w + b) ----
    taup = ctx.enter_context(tc.tile_pool(name="taup", bufs=H))
    tau_scs = []
    for h in range(H):
        part = smallp.tile([128, 1], F32, tag="part")
        nc.vector.reduce_sum(out=part, in_=std3[:, :, h], axis=X)
        mean_ps = ps_sm.tile([1, 1], F32, tag="ps_sm")
        nc.tensor.matmul(mean_ps, lhsT=inv_t, rhs=part, start=True, stop=True)
        ez = smallp.tile([1, 1], F32, tag="ez")
        nc.scalar.activation(ez, mean_ps, Exp, bias=negb[:], scale=negw[:])
        den = smallp.tile([1, 1], F32, tag="den")
        nc.vector.tensor_scalar_add(den, ez, 1.0)
        sig = smallp.tile([1, 1], F32, tag="sig")
        nc.vector.reciprocal(sig, den)
        tau_ps = ps_sm.tile([128, 1], F32, tag="ps_sm")
        nc.tensor.matmul(tau_ps, lhsT=bc2, rhs=sig, start=True, stop=True)
        tau_sc = taup.tile([128, 1], F32, tag="tau_sc")
        nc.vector.tensor_copy(tau_sc, tau_ps)
        tau_scs.append(tau_sc)

    for h in range(H):
        tau_sc = tau_scs[h]
        # ---- loads (natural [t_mod, tt, d] tiling) ----
        q_nat = nat.tile([128, TT, 128], F32, tag="q_nat")
        nc.sync.dma_start(out=q_nat, in_=q_d[:, h, :].rearrange("(tt p) d -> p tt d", p=128))
        k_nat = nat.tile([128, TT, 128], F32, tag="k_nat")
        nc.sync.dma_start(out=k_nat, in_=k_d[:, h, :].rearrange("(tt p) d -> p tt d", p=128))
        v_nat = nat.tile([128, TT, 128], F32R, tag="v_nat")
        nc.sync.dma_start(out=v_nat, in_=_r(v_d[:, h, :].rearrange("(tt p) d -> p tt d", p=128)))

        # ---- transposes: q packs stay in PSUM until mean is known, then are
        # evacuated on ACT with the centering fused in as Identity+bias.
        qcT = big.tile([128, T], F32R, tag="qcT")
        kT = big.tile([128, T], F32R, tag="kT")
        qpacks = []
        mups = []
        for a in range(TT // 4):
            pack = ps_sm.tile([128, 512], F32, tag="ps_sm")
            for j in range(4):
                nc.tensor.transpose(pack[:, j * 128:(j + 1) * 128], q_nat[:, a * 4 + j, :], ident)
            qpacks.append(pack)
            mup = smallp.tile([128, 1], F32, tag="mup%d" % a)
            nc.vector.reduce_sum(out=mup, in_=pack, axis=X)
            mups.append(mup)
        musum = smallp.tile([128, 1], F32, tag="musum")
        nc.vector.tensor_add(musum, mups[0], mups[1])
        nmu = smallp.tile([128, 1], F32, tag="nmu")
        nc.vector.tensor_scalar_mul(nmu, musum, -1.0 / T)
        for a in range(TT // 4):
            nc.scalar.activation(qcT[:, a * 512:(a + 1) * 512], qpacks[a], Ident,
                                 bias=nmu[:], scale=1.0)
        for a in range(TT // 4):
            pack = ps_sm.tile([128, 512], F32, tag="ps_sm")
            for j in range(4):
                nc.tensor.transpose(pack[:, j * 128:(j + 1) * 128], k_nat[:, a * 4 + j, :], ident)
            nc.vector.tensor_copy(kT[:, a * 512:(a + 1) * 512], pack)

        # ---- main s-tile loop ----
        ot_ps = ps_ot.tile([128, T], F32, tag="ps_ot")
        esum = esp.tile([128, T], F32, tag="esum")
        for i in range(TT):
            st_ps = ps_st.tile([128, T], F32, tag="ps_st")
            klhs = kT[:, i * 128:(i + 1) * 128]
            nc.tensor.matmul(st_ps[:, 0:512], lhsT=klhs, rhs=qcT[:, 0:512], start=True, stop=True)
            nc.tensor.matmul(st_ps[:, 512:1024], lhsT=klhs, rhs=qcT[:, 512:1024], start=True, stop=True)
            et = etp.tile([128, T], F32R, tag="et")
            nc.scalar.activation(et, st_ps, Exp, bias=0.0, scale=tau_sc[:])
            vlhs = v_nat[:, i, :]
            nc.tensor.matmul(ot_ps[:, 0:512], lhsT=vlhs, rhs=et[:, 0:512], start=(i == 0), stop=(i == TT - 1))
            nc.tensor.matmul(ot_ps[:, 512:1024], lhsT=vlhs, rhs=et[:, 512:1024], start=(i == 0), stop=(i == TT - 1))
            if i == 1:
                nc.vector.tensor_add(esum, prev_et.bitcast(F32), et.bitcast(F32))
            elif i > 1:
                nc.vector.tensor_add(esum, esum, et.bitcast(F32))
            prev_et = et

        # ---- rowsums (transposed orientation) + reciprocal ----
        rs_ps = ps_sm.tile([128, TT], F32, tag="ps_sm")
        for tt in range(TT):
            nc.tensor.matmul(rs_ps[:, tt:tt + 1], lhsT=esum[:, tt * 128:(tt + 1) * 128],
                             rhs=ones128, start=True, stop=True)
        recipT = smallp.tile([128, TT], F32, tag="recipT")
        nc.vector.reciprocal(recipT, rs_ps)

        # ---- evacuate O^T (ACT), transpose back, normalize ----
        ots = otsp.tile([128, T], F32, tag="ots")
        nc.scalar.copy(ots, ot_ps)
        o_nat = onatp.tile([128, TT, 128], F32, tag="o_nat")
        for a in range(TT // 4):
            fpack = ps_st.tile([128, T], F32, tag="ps_st")
            for j in range(4):
                tt = a * 4 + j
                nc.tensor.transpose(fpack[:, j * 128:(j + 1) * 128], ots[:, tt * 128:(tt + 1) * 128], ident)
            for j in range(4):
                tt = a * 4 + j
                nc.vector.tensor_scalar_mul(o_nat[:, tt, :], fpack[:, j * 128:(j + 1) * 128],
                                            recipT[:, tt:tt + 1])
        nc.sync.dma_start(out=o_d[:, h, :].rearrange("(tt p) d -> p tt d", p=128), in_=o_nat)

    ctx.close()


_BUILT = None


def _build():
    global _BUILT
    if _BUILT is None:
        nc = bacc.Bacc("TRN2", target_bir_lowering=False, debug=False, num_devices=None)
        with tile.TileContext(nc) as tc:
            _emit(tc)
        nc.compile()
        _BUILT = nc
    return _BUILT


def _in_maps(Q, K, V, std, tau_w, tau_b):
    tw = np.asarray(tau_w, np.float32).reshape(1, 1)
    tb = np.asarray(tau_b, np.float32).reshape(1, 1)
    maps = []
    for c in range(NCORES):
        b, n0 = c // 2, (c % 2) * H
        maps.append({
            "Q": np.ascontiguousarray(Q[b, :, n0:n0 + H, :], np.float32),
            "K": np.ascontiguousarray(K[b, :, n0:n0 + H, :], np.float32),
            "V": np.ascontiguousarray(V[b, :, n0:n0 + H, :], np.float32),
            "S": np.ascontiguousarray(std[b, :, n0:n0 + H, 0], np.float32),
            "TW": tw,
            "TB": tb,
        })
    return maps


def _gather(results):
    out = np.empty((B, T, N, D), np.float32)
    for c in range(NCORES):
        b, n0 = c // 2, (c % 2) * H
        out[b, :, n0:n0 + H, :] = results[c]["O"]
    return out


def run(Q, K, V, std, tau_w, tau_b, **spmd_kwargs):
    nc = _build()
    res = run_bass_kernel_spmd(nc, _in_maps(Q, K, V, std, tau_w, tau_b),
                               core_ids=list(range(NCORES)), **spmd_kwargs)
    return _gather(res.results), res


def kernel(Q, K, V, std, tau_w, tau_b):
    out, _ = run(Q, K, V, std, tau_w, tau_b)
    return out
